# revision 1
# baseline (speedup 1.0000x reference)
"""AddShift_mp_linear_module on 8 TRN2 NeuronCores.

Strategy (channel-block sharding, no collectives):
  - 96 output-channel blocks (11 input channels each) -> 12 blocks/core.
  - Every branch is a contraction over the block's (k, spatial) axis:
      out_v[co, h, (b,w)]  = sum_{k,h'} Ov[(k,h'), h]   * x[b, c, h', w]
      out_i[co, h, (b,w)]  = sum_{k,h'} Oi[(k,h'), h]   * x[b, c, h', w]
      out_h[co, w, (b,h')] = sum_{k,w'} Oh[(k,w'), w]   * x[b, c, h', w']
    where the sparse operators Ov/Oi/Oh are built on the host from
    w1/w2/w3/pad_hv/idx_identit (all known at call time).
  - Precision split: x rides the wire as fp8 e3m4 (1.3% RMS on randn,
    half the bytes of bf16 -- x dominates DMA), operators stay bf16
    (mixed-dtype matmul), outputs bf16. Total rel err ~1.5e-2.
  - On device: per block, two interleaved PSUM-accumulation chains of
    6 matmuls each over [110,448] K-chunks (660 rows = 6 x 110, exact,
    no tail). V and identity share one chain (stationary [110,120],
    identity at cols 64:120 for 32-aligned PSUM reads); H uses a host-
    pretransposed w-major copy of x. A warmup burst of dummy matmuls on
    a memset tile (no DMA dependency) ramps the PE clock during DMA
    startup.
  - Per block: one x DMA split in two halves (earlier compute start),
    one operator DMA on the opposite HWDGE ring, one output DMA.
  - Outputs leave as [56, 3, 448] bf16 tiles; host restores
    (out_h, out_v, out_i) [b, co, h, w] fp32.
"""

import numpy as np
import ml_dtypes

# architecture constants (match reference init_kwargs)
B = 8
C_OUT = 96
NK = 11
G = 4
C_IN = C_OUT * NK          # 1056
HOUT = WOUT = 56
HIN = WIN = 60
EP = 2                     # extra pad
N_CORES = 8
BPC = C_OUT // N_CORES     # blocks per core = 12
CPC = BPC * NK             # channels per core = 132
KROWS = NK * HIN           # 660 real contraction rows per block
KP = 110                   # partitions per chunk
NJ = 6                     # chunks: 6 x 110 = 660, exact
NJH = 3                    # first-half chunks (earlier compute start)
NFREE = B * WOUT           # 448 matmul free dim (w/h pre-sliced to [2,58))
MOP = 120 + 56             # operator cols: V+identity 120 | H 56
N_WARM = 4                 # PE warmup matmuls: block-0 x now arrives ~9us
                           # (finer split), so a short ramp burst suffices

F8 = ml_dtypes.float8_e3m4
BF16 = ml_dtypes.bfloat16

_CACHE = {}


def _build_operators(w1, w2, w3, pad_hv, idx_identit):
    """Build per-block stationary operators.

    Returns opv (96, 660, 120) fp32  [cols 0:56 = V, 64:120 = identity]
            oph (96, 660, 56)  fp32
    Row r = k*60 + spatial_in, for channel c = co*11 + k.
    """
    w1r = np.asarray(w1, np.float32).reshape(G, C_IN)
    w2r = np.asarray(w2, np.float32).reshape(G, C_IN)
    w3r = np.asarray(w3, np.float32).reshape(G, C_OUT)
    pad = np.asarray(pad_hv, np.int64)            # (C_IN, 2G)
    idx = np.asarray(idx_identit, np.int64)       # (C_OUT, G)

    opv = np.zeros((C_OUT, KROWS, 120), np.float32)
    oph = np.zeros((C_OUT, KROWS, 56), np.float32)

    c_all = np.arange(C_IN)
    co_all = c_all // NK
    k_all = c_all % NK
    pos = np.arange(HOUT)                          # output spatial index

    for g in range(G):
        # horizontal: w_in = w_out + EP + pad[c, g]
        win = pos[None, :] + EP + pad[:, g][:, None]        # (C_IN, 56)
        ok = (win >= 0) & (win < WIN)
        cc, oo = np.nonzero(ok)
        np.add.at(oph, (co_all[cc], k_all[cc] * HIN + win[cc, oo], oo), w1r[g, cc])
        # vertical: h_in = h_out + EP + pad[c, G+g]
        hin = pos[None, :] + EP + pad[:, G + g][:, None]
        ok = (hin >= 0) & (hin < HIN)
        cc, oo = np.nonzero(ok)
        np.add.at(opv, (co_all[cc], k_all[cc] * HIN + hin[cc, oo], oo), w2r[g, cc])

    # identity: out_i[co] = sum_g w3r[g, co] * x[idx[co, g]] (idx within block co)
    k_sel = idx - np.arange(C_OUT)[:, None] * NK            # (C_OUT, G)
    assert np.all((k_sel >= 0) & (k_sel < NK)), "idx_identit outside its block"
    u = np.zeros((C_OUT, NK), np.float32)
    for g in range(G):
        np.add.at(u, (np.arange(C_OUT), k_sel[:, g]), w3r[g])
    co_i, k_i = np.nonzero(u != 0)
    for co, k in zip(co_i, k_i):
        opv[co, k * HIN + pos + EP, 64 + pos] += u[co, k]
    return opv, oph


def _build_nc():
    import concourse.bacc as bacc
    import concourse.tile as tile
    import concourse.bass as bass
    import concourse.mybir as mybir
    from contextlib import ExitStack

    f32 = mybir.dt.float32
    f8 = mybir.dt.float8e3
    bf16 = mybir.dt.bfloat16

    nc = bacc.Bacc(None, target_bir_lowering=False)
    # x both orientations interleaved: [bi, p, j, o(orient), n]
    # row r = j*110 + p  for chunk j, partition p
    xm_d = nc.declare_dram_parameter(
        "xmain", [BPC, KP, NJ, 2, NFREE], f8, isOutput=False)
    # operators, partition-major: [p, bi, j, m] (V 0:120 | H 120:176)
    op_d = nc.declare_dram_parameter(
        "ops", [KP, BPC, NJ, MOP], f8, isOutput=False)
    out_d = nc.declare_dram_parameter("out", [BPC, 56, 3, NFREE], bf16, isOutput=True)

    with tile.TileContext(nc) as tc, ExitStack() as ctx:
        rhs_pool = ctx.enter_context(tc.tile_pool(name="rhs", bufs=4))
        op_pool = ctx.enter_context(tc.tile_pool(name="ops", bufs=1))
        o_pool = ctx.enter_context(tc.tile_pool(name="outs", bufs=3))
        psum_pool = ctx.enter_context(
            tc.tile_pool(name="psum", bufs=4, space=bass.MemorySpace.PSUM)
        )
        # warmup tiles: memset-only (no DMA dependency) so the PE ramps
        # toward full clock while the first block's DMA is in flight; sized
        # to end roughly when that DMA lands
        warm = op_pool.tile([KP, NFREE], f8, tag="warm")
        nc.vector.memset(warm[:], 0)
        pw = psum_pool.tile([120, NFREE], f32, tag="pv")
        for _ in range(N_WARM):
            nc.tensor.matmul(pw[:], warm[:, :120], warm[:], start=True, stop=True)
        for bi in range(BPC):
            # x first (critical path), operators on the opposite HWDGE ring
            xe = nc.sync if bi % 2 == 0 else nc.scalar
            oe = nc.scalar if bi % 2 == 0 else nc.sync
            # split x so early chunks' matmuls start while the rest is in
            # flight; block 0 splits finest (it gates the whole pipeline)
            xparts = [1, 2, 3] if bi == 0 else [NJH, NJ - NJH]
            xtiles, j0 = [], 0
            for pi, np_ in enumerate(xparts):
                xt = rhs_pool.tile([KP, np_, 2, NFREE], f8, tag=f"xt{pi}_{np_}")
                xe.dma_start(xt[:], xm_d[bi, :, j0:j0 + np_])
                xtiles.append((j0, np_, xt))
                j0 += np_
            oparts = [1, NJ - 1] if bi == 0 else [NJ]
            otiles, j0 = [], 0
            for pi, np_ in enumerate(oparts):
                ot = op_pool.tile([KP, np_, MOP], f8, tag=f"op{bi}_{pi}")
                oe.dma_start(ot[:], op_d[:, bi, j0:j0 + np_])
                otiles.append((j0, np_, ot))
                j0 += np_
            psum_vi = psum_pool.tile([120, NFREE], f32, tag="pv")
            psum_h = psum_pool.tile([56, NFREE], f32, tag="ph")
            # interleave the two accumulation chains so PE drains overlap
            for j in range(NJ):
                xt = next(t for (s, n, t) in xtiles if s <= j < s + n)
                xs = next(s for (s, n, t) in xtiles if s <= j < s + n)
                ot = next(t for (s, n, t) in otiles if s <= j < s + n)
                os_ = next(s for (s, n, t) in otiles if s <= j < s + n)
                nc.tensor.matmul(
                    psum_vi[:], ot[:, j - os_, :120], xt[:, j - xs, 0, :],
                    start=(j == 0), stop=(j == NJ - 1),
                )
                nc.tensor.matmul(
                    psum_h[:], ot[:, j - os_, 120:], xt[:, j - xs, 1, :],
                    start=(j == 0), stop=(j == NJ - 1),
                )
            # stage outputs: V+I leave as soon as their copies land; H (whose
            # chain finishes last) follows on the idle gpsimd ring
            st1 = o_pool.tile([56, 2, NFREE], bf16, tag="st1")
            st2 = o_pool.tile([56, NFREE], bf16, tag="st2")
            nc.scalar.copy(st1[:, 0, :], psum_vi[:56])
            nc.vector.tensor_copy(st1[:, 1, :], psum_vi[64:120])
            oe.dma_start(out_d[bi, :, :2], st1[:])
            nc.vector.tensor_copy(st2[:], psum_h[:])
            # last block's H rides the op ring: gpsimd's slow SWDGE drain
            # then completes before the tail instead of inside it
            he = oe if bi == BPC - 1 else nc.gpsimd
            he.dma_start(out_d[bi, :, 2], st2[:])
    nc.finalize()
    return nc


def prepare_inputs(x, w1, w2, w3, pad_hv, idx_identit):
    """Host-side shard prep. Returns in_maps (list of 8 dicts)."""
    x = np.asarray(x)
    xb = x.astype(F8)                                     # (B, C, 60, 60)
    # h-major for V/I: [c, h', (b, w in [2,58))]
    x_hbw = np.ascontiguousarray(
        xb[:, :, :, EP:EP + WOUT].transpose(1, 2, 0, 3)).reshape(C_IN * HIN, NFREE)
    # w-major for H: [c, w', (b, h in [2,58))]
    x_wbh = np.ascontiguousarray(
        xb[:, :, EP:EP + HOUT, :].transpose(1, 3, 0, 2)).reshape(C_IN * WIN, NFREE)

    opv, oph = _build_operators(w1, w2, w3, pad_hv, idx_identit)
    OP = np.concatenate([opv, oph], axis=2).astype(F8)     # (96, 660, 176)

    in_maps = []
    for i in range(N_CORES):
        r0 = i * CPC * HIN
        xv = x_hbw[r0:r0 + CPC * HIN].reshape(BPC, KROWS, NFREE)
        xh = x_wbh[r0:r0 + CPC * WIN].reshape(BPC, KROWS, NFREE)
        arr = np.stack([xv, xh], axis=2)                   # (BPC, 660, 2o, F)
        arr = arr.reshape(BPC, NJ, KP, 2, NFREE)           # (BPC, j, p, o, F)
        xmain = np.ascontiguousarray(arr.transpose(0, 2, 1, 3, 4))
        opc = OP[i * BPC:(i + 1) * BPC].reshape(BPC, NJ, KP, MOP)
        ops = np.ascontiguousarray(opc.transpose(2, 0, 1, 3))  # (p, bi, j, m)
        in_maps.append({"xmain": xmain, "ops": ops})
    return in_maps


def unshard(results):
    """results: list of 8 dicts with 'out' (BPC, 56, 3, 448) bf16 ->
    (out_h, out_v, out_i) each (B, C_OUT, 56, 56) fp32."""
    O = np.stack([np.asarray(r["out"], np.float32) for r in results])  # (8,12,56,3,448)
    O = O.reshape(N_CORES, BPC, 56, 3, B, WOUT)
    # (core, co_l, h, b, w) -> (b, core, co_l, h, w)
    out_v = O[:, :, :, 0].transpose(3, 0, 1, 2, 4).reshape(B, C_OUT, HOUT, WOUT)
    out_i = O[:, :, :, 1].transpose(3, 0, 1, 2, 4).reshape(B, C_OUT, HOUT, WOUT)
    h = O[:, :, :, 2]                          # (core, co_l, w, b, h)
    out_h = h.transpose(3, 0, 1, 4, 2).reshape(B, C_OUT, HOUT, WOUT)
    return out_h, out_v, out_i


def kernel(x, w1, w2, w3, pad_hv, idx_identit, b=B, hout=HOUT, wout=WOUT):
    from concourse.bass_utils import run_bass_kernel_spmd

    assert int(b) == B and int(hout) == HOUT and int(wout) == WOUT
    assert tuple(np.asarray(x).shape) == (B, C_IN, HIN, WIN)

    in_maps = prepare_inputs(x, w1, w2, w3, pad_hv, idx_identit)
    nc = _CACHE.get("nc")
    if nc is None:
        nc = _build_nc()
        _CACHE["nc"] = nc
    res = run_bass_kernel_spmd(nc, in_maps, core_ids=list(range(N_CORES)))
    return unshard(res.results)



# revision 4
# speedup vs baseline: 1.3642x; 1.3642x over previous
"""AddShift_mp_linear_module on 8 TRN2 NeuronCores — v2 (column-tiled PE).

Strategy (channel-block sharding, no collectives):
  - 96 output-channel blocks (11 input channels each) -> 12 blocks/core,
    processed as 6 block-pairs.
  - Each branch is a dense contraction per block:
      V:  out_v[h, (b,w)]  = sum_{(k,h')} Ov[(k,h'), h] * x[c, h', (b,w)]
      H:  out_hT[w, (b,h)] = sum_{(k,w')} Oh[(k,w'), w] * x[c, w', (b,h)]
      I:  out_i[h, (b,w)]  = sum_{(g,h')} u_g*[h'==h]   * x[c_g, h', (b,w)]
    All three have M=56 output columns, so TWO chains run CONCURRENTLY on
    the PE via column tiling: tile_position (0,0) and (0,64) stream their
    own moving operands into separate 64-column halves of the array
    (measured: pair span == single-matmul span, 2x throughput).
  - Per pair: V_e||H_e (6 chunk spans), V_o||H_o (6), I_e||I_o (2 spans,
    identity rows gathered host-side into an aligned [112,2,448] tile).
    84 spans/core at ~190 ns warm ≈ 16 us PE, hidden under DMA.
  - Precision: x and operators ride as fp8 e3m4 (same as the math that
    measures rel_err 1.85e-2), outputs bf16.
  - DMA: ~11.3 MB/core in ~50 large transfers; operators prefetched up
    front on the scalar ring, x pair-level on the sync ring, outputs on
    the scalar ring as they are produced.
"""

import numpy as np
import ml_dtypes

# architecture constants (match reference init_kwargs)
B = 8
C_OUT = 96
NK = 11
G = 4
C_IN = C_OUT * NK          # 1056
HOUT = WOUT = 56
HIN = WIN = 60
EP = 2                     # extra pad
N_CORES = 8
BPC = C_OUT // N_CORES     # blocks per core = 12
CPC = BPC * NK             # channels per core = 132
NPAIR = BPC // 2           # block pairs per core = 6
KROWS = NK * HIN           # 660 contraction rows per block (V/H)
KCH = 110                  # chunk rows
NJ = 6                     # chunks: 6 x 110 = 660 exact
NFREE = B * WOUT           # 448 matmul free dim
N_WARM = 12                # PE warmup matmuls while first DMAs fly

F8 = ml_dtypes.float8_e3m4

_CACHE = {}


def _build_vh_operators(w1, w2, pad_hv):
    """Dense V/H operators per block: (C_OUT, 660, 56) fp32 each.
    Row r = k*60 + spatial_in for channel c = co*11 + k."""
    w1r = np.asarray(w1, np.float32).reshape(G, C_IN)
    w2r = np.asarray(w2, np.float32).reshape(G, C_IN)
    pad = np.asarray(pad_hv, np.int64)            # (C_IN, 2G)
    opv = np.zeros((C_OUT, KROWS, 56), np.float32)
    oph = np.zeros((C_OUT, KROWS, 56), np.float32)
    c_all = np.arange(C_IN)
    co_all = c_all // NK
    k_all = c_all % NK
    pos = np.arange(HOUT)
    for g in range(G):
        win = pos[None, :] + EP + pad[:, g][:, None]        # (C_IN, 56)
        ok = (win >= 0) & (win < WIN)
        cc, oo = np.nonzero(ok)
        np.add.at(oph, (co_all[cc], k_all[cc] * HIN + win[cc, oo], oo), w1r[g, cc])
        hin = pos[None, :] + EP + pad[:, G + g][:, None]
        ok = (hin >= 0) & (hin < HIN)
        cc, oo = np.nonzero(ok)
        np.add.at(opv, (co_all[cc], k_all[cc] * HIN + hin[cc, oo], oo), w2r[g, cc])
    return opv, oph


def _identity_slots(w3, idx_identit):
    """Per block: up to 4 (k, coeff) identity terms, padded to 4 slots."""
    w3r = np.asarray(w3, np.float32).reshape(G, C_OUT)
    idx = np.asarray(idx_identit, np.int64)       # (C_OUT, G)
    k_sel = idx - np.arange(C_OUT)[:, None] * NK
    assert np.all((k_sel >= 0) & (k_sel < NK))
    u = np.zeros((C_OUT, NK), np.float32)
    for g in range(G):
        np.add.at(u, (np.arange(C_OUT), k_sel[:, g]), w3r[g])
    ks = np.zeros((C_OUT, 4), np.int64)
    cf = np.zeros((C_OUT, 4), np.float32)
    for co in range(C_OUT):
        nz = np.nonzero(u[co])[0]
        ks[co, :len(nz)] = nz
        cf[co, :len(nz)] = u[co, nz]
    return ks, cf


def _build_nc():
    import concourse.bacc as bacc
    import concourse.tile as tile
    import concourse.bass as bass
    import concourse.mybir as mybir
    from contextlib import ExitStack

    f32 = mybir.dt.float32
    f8 = mybir.dt.float8e3
    bf16 = mybir.dt.bfloat16

    nc = bacc.Bacc(None, target_bir_lowering=False)
    xh_d = nc.declare_dram_parameter(
        "xh", [NPAIR, KCH, 2, NJ, NFREE], f8, isOutput=False)
    xw_d = nc.declare_dram_parameter(
        "xw", [NPAIR, KCH, 2, NJ, NFREE], f8, isOutput=False)
    xi_d = nc.declare_dram_parameter(
        "xi", [NPAIR, 112, 2, 2, NFREE], f8, isOutput=False)
    opvh_d = nc.declare_dram_parameter(
        "opvh", [KCH, BPC, NJ, 112], f8, isOutput=False)
    opi_d = nc.declare_dram_parameter(
        "opi", [112, BPC, 2, 56], f8, isOutput=False)
    out1_d = nc.declare_dram_parameter("out1", [BPC, 120, NFREE], bf16,
                                       isOutput=True)
    out2_d = nc.declare_dram_parameter("out2", [NPAIR, 120, NFREE], bf16,
                                       isOutput=True)

    with tile.TileContext(nc) as tc, ExitStack() as ctx:
        xpool = ctx.enter_context(tc.tile_pool(name="xp", bufs=1))
        oppool = ctx.enter_context(tc.tile_pool(name="opp", bufs=1))
        spool = ctx.enter_context(tc.tile_pool(name="stg", bufs=4))
        wpool = ctx.enter_context(tc.tile_pool(name="wp", bufs=1))
        psum_pool = ctx.enter_context(
            tc.tile_pool(name="psum", bufs=2, space=bass.MemorySpace.PSUM))
        wppool = ctx.enter_context(
            tc.tile_pool(name="wpp", bufs=1, space=bass.MemorySpace.PSUM))

        # ---- operator prefetch (scalar ring) + x DMAs (sync ring) ----
        op_ts, oi_ts = [], []
        xh_ts, xw_ts, xi_ts = [], [], []
        for q in range(NPAIR):
            op_t = oppool.tile([KCH, 2, NJ, 112], f8, tag=f"op{q}", name=f"op{q}")
            oi_t = oppool.tile([112, 2, 2, 56], f8, tag=f"oi{q}", name=f"oi{q}")
            nc.scalar.dma_start(op_t[:], opvh_d[:, 2 * q:2 * q + 2])
            nc.scalar.dma_start(oi_t[:], opi_d[:, 2 * q:2 * q + 2])
            op_ts.append(op_t)
            oi_ts.append(oi_t)
        for q in range(NPAIR):
            xh_t = xpool.tile([KCH, 2, NJ, NFREE], f8, tag=f"xh{q}", name=f"xh{q}")
            xw_t = xpool.tile([KCH, 2, NJ, NFREE], f8, tag=f"xw{q}", name=f"xw{q}")
            xi_t = xpool.tile([112, 2, 2, NFREE], f8, tag=f"xi{q}", name=f"xi{q}")
            nc.sync.dma_start(xh_t[:], xh_d[q])
            nc.sync.dma_start(xw_t[:], xw_d[q])
            nc.sync.dma_start(xi_t[:], xi_d[q])
            xh_ts.append(xh_t)
            xw_ts.append(xw_t)
            xi_ts.append(xi_t)

        # ---- PE warmup on memset tiles (no DMA dependency) ----
        warm = wpool.tile([KCH, NFREE], f8, tag="warm")
        wst = wpool.tile([KCH, 56], f8, tag="wst")
        nc.vector.memset(warm[:], 0)
        nc.vector.memset(wst[:], 0)
        pw = wppool.tile([128, NFREE], f32, tag="pw")
        for w in range(N_WARM):
            pos = (0, 0) if w % 2 == 0 else (0, 64)
            dst = pw[0:56] if w % 2 == 0 else pw[64:120]
            nc.tensor.matmul(dst, wst[:], warm[:], start=True, stop=True,
                             tile_position=pos)

        # ---- main: 6 block pairs ----
        for q in range(NPAIR):
            xh_t, xw_t, xi_t = xh_ts[q], xw_ts[q], xi_ts[q]
            op_t, oi_t = op_ts[q], oi_ts[q]
            pvh = [psum_pool.tile([128, NFREE], f32, tag="pe", name=f"pe{q}"),
                   psum_pool.tile([128, NFREE], f32, tag="po", name=f"po{q}")]
            pi = psum_pool.tile([128, NFREE], f32, tag="pi", name=f"pi{q}")
            for b in (0, 1):
                pt = pvh[b]
                for j in range(NJ):
                    nc.tensor.matmul(pt[0:56], op_t[:, b, j, 0:56],
                                     xh_t[:, b, j, :],
                                     start=(j == 0), stop=(j == NJ - 1),
                                     tile_position=(0, 0))
                    nc.tensor.matmul(pt[64:120], op_t[:, b, j, 56:112],
                                     xw_t[:, b, j, :],
                                     start=(j == 0), stop=(j == NJ - 1),
                                     tile_position=(0, 64))
            for c in range(2):
                nc.tensor.matmul(pi[0:56], oi_t[:, 0, c, :], xi_t[:, 0, c, :],
                                 start=(c == 0), stop=(c == 1),
                                 tile_position=(0, 0))
                nc.tensor.matmul(pi[64:120], oi_t[:, 1, c, :], xi_t[:, 1, c, :],
                                 start=(c == 0), stop=(c == 1),
                                 tile_position=(0, 64))
            # drain psums -> bf16 staging -> DRAM (scalar ring)
            for b in (0, 1):
                stg = spool.tile([120, NFREE], bf16, tag=f"sg{b}", name=f"sg{q}_{b}")
                if b == 0:
                    nc.scalar.copy(stg[:], pvh[b][0:120])
                else:
                    nc.vector.tensor_copy(stg[:], pvh[b][0:120])
                nc.scalar.dma_start(out1_d[2 * q + b], stg[:])
            sti = spool.tile([120, NFREE], bf16, tag="sgi", name=f"sgi{q}")
            nc.vector.tensor_copy(sti[0:56], pi[0:56])
            nc.scalar.copy(sti[64:120], pi[64:120])
            nc.scalar.dma_start(out2_d[q], sti[:])
    nc.finalize()
    return nc


def prepare_inputs(x, w1, w2, w3, pad_hv, idx_identit):
    """Host-side shard prep. Returns in_maps (list of 8 dicts)."""
    x = np.asarray(x)
    xq = x.astype(F8)                                     # (B, C, 60, 60)
    opv, oph = _build_vh_operators(w1, w2, pad_hv)        # (96, 660, 56) f32
    ks, cf = _identity_slots(w3, idx_identit)             # (96,4) each

    in_maps = []
    for i in range(N_CORES):
        blocks = np.arange(i * BPC, (i + 1) * BPC)
        csl = slice(i * CPC, (i + 1) * CPC)
        # h-major: [blk, (k,h'), (b,w)] -> [pair, 110, blk2, chunk, 448]
        ch = xq[:, csl, :, EP:EP + WOUT]                   # (8, 132, 60, 56)
        ch = ch.transpose(1, 2, 0, 3).reshape(BPC, KROWS, NFREE)
        xh = np.ascontiguousarray(
            ch.reshape(NPAIR, 2, NJ, KCH, NFREE).transpose(0, 3, 1, 2, 4))
        # w-major: [blk, (k,w'), (b,h)]
        cw = xq[:, csl, EP:EP + HOUT, :]                   # (8, 132, 56, 60)
        cw = cw.transpose(1, 3, 0, 2).reshape(BPC, KROWS, NFREE)
        xw = np.ascontiguousarray(
            cw.reshape(NPAIR, 2, NJ, KCH, NFREE).transpose(0, 3, 1, 2, 4))
        # identity gather: [blk, slot, h', (b,w)] -> [pair, 112, blk2, chunk, 448]
        chan = (blocks[:, None] * NK + ks[blocks]).astype(np.int64)  # (12,4)
        xi_raw = xq[:, chan, EP:EP + HOUT, EP:EP + WOUT]   # (8, 12, 4, 56, 56)
        xi_raw = xi_raw.transpose(1, 2, 3, 0, 4).reshape(BPC, 224, NFREE)
        xi = np.ascontiguousarray(
            xi_raw.reshape(NPAIR, 2, 2, 112, NFREE).transpose(0, 3, 1, 2, 4))
        # V/H operators: [110, blk, chunk, 112]
        opv_c = opv[blocks].reshape(BPC, NJ, KCH, 56)
        oph_c = oph[blocks].reshape(BPC, NJ, KCH, 56)
        opvh = np.concatenate([opv_c, oph_c], axis=3)      # (12, 6, 110, 112)
        opvh = np.ascontiguousarray(
            opvh.transpose(2, 0, 1, 3)).astype(F8)         # (110, 12, 6, 112)
        # identity operators: [112, blk, chunk, 56] (scaled identities)
        eye = np.eye(56, dtype=np.float32)
        tmp = cf[blocks].reshape(BPC, 2, 2)[:, :, :, None, None] * eye
        opi = np.ascontiguousarray(
            tmp.transpose(2, 3, 0, 1, 4).reshape(112, BPC, 2, 56)).astype(F8)
        in_maps.append({"xh": xh, "xw": xw, "xi": xi, "opvh": opvh, "opi": opi})
    return in_maps


def unshard(results):
    """-> (out_h, out_v, out_i) each (B, C_OUT, 56, 56) fp32."""
    o1 = np.stack([np.asarray(r["out1"], np.float32) for r in results])
    o2 = np.stack([np.asarray(r["out2"], np.float32) for r in results])
    V = o1[:, :, 0:56].reshape(N_CORES, BPC, 56, B, WOUT)
    out_v = V.transpose(3, 0, 1, 2, 4).reshape(B, C_OUT, HOUT, WOUT)
    Hh = o1[:, :, 64:120].reshape(N_CORES, BPC, 56, B, HOUT)  # [.., w, b, h]
    out_h = Hh.transpose(3, 0, 1, 4, 2).reshape(B, C_OUT, HOUT, WOUT)
    Ie = o2[:, :, 0:56].reshape(N_CORES, NPAIR, 56, B, WOUT)
    Io = o2[:, :, 64:120].reshape(N_CORES, NPAIR, 56, B, WOUT)
    I2 = np.stack([Ie, Io], axis=2)          # [core, pair, half, h, b, w]
    out_i = I2.transpose(4, 0, 1, 2, 3, 5).reshape(B, C_OUT, HOUT, WOUT)
    return out_h, out_v, out_i


def kernel(x, w1, w2, w3, pad_hv, idx_identit, b=B, hout=HOUT, wout=WOUT):
    from concourse.bass_utils import run_bass_kernel_spmd

    assert int(b) == B and int(hout) == HOUT and int(wout) == WOUT
    assert tuple(np.asarray(x).shape) == (B, C_IN, HIN, WIN)

    in_maps = prepare_inputs(x, w1, w2, w3, pad_hv, idx_identit)
    nc = _CACHE.get("nc")
    if nc is None:
        nc = _build_nc()
        _CACHE["nc"] = nc
    res = run_bass_kernel_spmd(nc, in_maps, core_ids=list(range(N_CORES)))
    return unshard(res.results)


# revision 6
# speedup vs baseline: 1.9957x; 1.4629x over previous
"""AddShift_mp_linear_module on 8 TRN2 NeuronCores — v3 (column-tiled PE,
DMA-balanced 128-partition tiles).

Strategy (channel-block sharding, no collectives):
  - 96 output-channel blocks (11 input channels each) -> 12 blocks/core,
    processed as 6 block-pairs.
  - Per block, three dense contractions, all with M=56 output columns:
      V:  out_v[h, (b,w)]  over (k,h') rows   (h-major x)
      H:  out_hT[w, (b,h)] over (k,w') rows   (w-major x)
      I:  out_i[h, (b,w)]  over gathered identity rows
    TWO chains run CONCURRENTLY on the PE via column tiling
    (tile_position (0,0) / (0,64)); measured pair span == single span.
  - Contraction rows per block: 660 = 5x128 + 20. Chunks are K=128 so the
    x DMAs use all 128 partitions (all 16 SDMA engines); the 20-row tails
    of all blocks ship as one small transfer.
  - Per pair: V_e||H_e (6 spans), V_o||H_o (6), I_e||I_o (2 spans).
    84 spans/core at ~190 ns warm; DMA (~10.8 MB/core) is the roofline.
  - Precision: fp8 e3m4 in, bf16 out (rel_err 1.85e-2 < 2e-2).
"""

import numpy as np
import ml_dtypes

# architecture constants (match reference init_kwargs)
B = 8
C_OUT = 96
NK = 11
G = 4
C_IN = C_OUT * NK          # 1056
HOUT = WOUT = 56
HIN = WIN = 60
EP = 2                     # extra pad
N_CORES = 8
BPC = C_OUT // N_CORES     # blocks per core = 12
CPC = BPC * NK             # channels per core = 132
NPAIR = BPC // 2           # block pairs per core = 6
KROWS = NK * HIN           # 660 contraction rows per block (V/H)
KM = 128                   # main chunk rows
NJM = 5                    # main chunks (5 x 128 = 640)
KT = KROWS - NJM * KM      # tail chunk rows = 20
NFREE = B * WOUT           # 448 matmul free dim
N_WARM = 12                # PE warmup matmuls while first DMAs fly

F8 = ml_dtypes.float8_e3m4

_CACHE = {}


def _build_vh_operators(w1, w2, pad_hv):
    """Dense V/H operators per block: (C_OUT, 660, 56) fp32 each.
    Row r = k*60 + spatial_in for channel c = co*11 + k."""
    w1r = np.asarray(w1, np.float32).reshape(G, C_IN)
    w2r = np.asarray(w2, np.float32).reshape(G, C_IN)
    pad = np.asarray(pad_hv, np.int64)            # (C_IN, 2G)
    opv = np.zeros((C_OUT, KROWS, 56), np.float32)
    oph = np.zeros((C_OUT, KROWS, 56), np.float32)
    c_all = np.arange(C_IN)
    co_all = c_all // NK
    k_all = c_all % NK
    pos = np.arange(HOUT)
    for g in range(G):
        win = pos[None, :] + EP + pad[:, g][:, None]        # (C_IN, 56)
        ok = (win >= 0) & (win < WIN)
        cc, oo = np.nonzero(ok)
        np.add.at(oph, (co_all[cc], k_all[cc] * HIN + win[cc, oo], oo), w1r[g, cc])
        hin = pos[None, :] + EP + pad[:, G + g][:, None]
        ok = (hin >= 0) & (hin < HIN)
        cc, oo = np.nonzero(ok)
        np.add.at(opv, (co_all[cc], k_all[cc] * HIN + hin[cc, oo], oo), w2r[g, cc])
    return opv, oph


def _identity_slots(w3, idx_identit):
    """Per block: up to 4 (k, coeff) identity terms, padded to 4 slots."""
    w3r = np.asarray(w3, np.float32).reshape(G, C_OUT)
    idx = np.asarray(idx_identit, np.int64)       # (C_OUT, G)
    k_sel = idx - np.arange(C_OUT)[:, None] * NK
    assert np.all((k_sel >= 0) & (k_sel < NK))
    u = np.zeros((C_OUT, NK), np.float32)
    for g in range(G):
        np.add.at(u, (np.arange(C_OUT), k_sel[:, g]), w3r[g])
    ks = np.zeros((C_OUT, 4), np.int64)
    cf = np.zeros((C_OUT, 4), np.float32)
    for co in range(C_OUT):
        nz = np.nonzero(u[co])[0]
        ks[co, :len(nz)] = nz
        cf[co, :len(nz)] = u[co, nz]
    return ks, cf


def _build_nc():
    import concourse.bacc as bacc
    import concourse.tile as tile
    import concourse.bass as bass
    import concourse.mybir as mybir
    from contextlib import ExitStack

    f32 = mybir.dt.float32
    f8 = mybir.dt.float8e3
    bf16 = mybir.dt.bfloat16

    nc = bacc.Bacc(None, target_bir_lowering=False)
    # main x: [pair, p, orient, blk, chunk, n] — one 1.15 MB DMA per pair
    xm_d = nc.declare_dram_parameter(
        "xm", [NPAIR, KM, 2, 2, NJM, NFREE], f8, isOutput=False)
    # x tails (rows 640:660 of each block/orientation), one DMA for all
    xt_d = nc.declare_dram_parameter(
        "xt", [KT, NPAIR, 2, 2, NFREE], f8, isOutput=False)
    # identity-gathered x: [pair, p, blk, chunk, n]
    xi_d = nc.declare_dram_parameter(
        "xi", [NPAIR, KM, 2, 2, NFREE], f8, isOutput=False)
    # operators
    opm_d = nc.declare_dram_parameter(
        "opm", [KM, BPC, NJM, 112], f8, isOutput=False)
    opt_d = nc.declare_dram_parameter(
        "opt", [KT, BPC, 112], f8, isOutput=False)
    opi_d = nc.declare_dram_parameter(
        "opi", [KM, BPC, 2, 56], f8, isOutput=False)
    # output: per pair [120, 3, 448] (cols: blk_e VH | blk_o VH | identity)
    out_d = nc.declare_dram_parameter(
        "out", [NPAIR, 120, 3, NFREE], bf16, isOutput=True)

    with tile.TileContext(nc) as tc, ExitStack() as ctx:
        xpool = ctx.enter_context(tc.tile_pool(name="xp", bufs=1))
        oppool = ctx.enter_context(tc.tile_pool(name="opp", bufs=1))
        spool = ctx.enter_context(tc.tile_pool(name="stg", bufs=4))
        wpool = ctx.enter_context(tc.tile_pool(name="wp", bufs=1))
        psum_pool = ctx.enter_context(
            tc.tile_pool(name="psum", bufs=2, space=bass.MemorySpace.PSUM))
        wppool = ctx.enter_context(
            tc.tile_pool(name="wpp", bufs=1, space=bass.MemorySpace.PSUM))

        # ---- ring A (scalar): operators + tails first, outputs later ----
        opm_t = oppool.tile([KM, BPC, NJM, 112], f8, tag="opm")
        opt_t = oppool.tile([KT, BPC, 112], f8, tag="opt")
        opi_t = oppool.tile([KM, BPC, 2, 56], f8, tag="opi")
        xt_t = oppool.tile([KT, NPAIR, 2, 2, NFREE], f8, tag="xt")
        nc.scalar.dma_start(opm_t[:, 0:2], opm_d[:, 0:2])
        nc.scalar.dma_start(opi_t[:, 0:2], opi_d[:, 0:2])
        nc.scalar.dma_start(opt_t[:], opt_d[:])
        nc.scalar.dma_start(xt_t[:], xt_d[:])
        nc.scalar.dma_start(opm_t[:, 2:BPC], opm_d[:, 2:BPC])
        nc.scalar.dma_start(opi_t[:, 2:BPC], opi_d[:, 2:BPC])

        # ---- ring S (sync): x main + identity x, pair by pair ----
        xm_ts, xi_ts = [], []
        for q in range(NPAIR):
            xm_t = xpool.tile([KM, 2, 2, NJM, NFREE], f8, tag=f"xm{q}",
                              name=f"xm{q}")
            xi_t = xpool.tile([KM, 2, 2, NFREE], f8, tag=f"xi{q}",
                              name=f"xi{q}")
            nc.sync.dma_start(xm_t[:], xm_d[q])
            nc.sync.dma_start(xi_t[:], xi_d[q])
            xm_ts.append(xm_t)
            xi_ts.append(xi_t)

        # ---- PE warmup on memset tiles (no DMA dependency) ----
        warm = wpool.tile([KM, NFREE], f8, tag="warm")
        wst = wpool.tile([KM, 56], f8, tag="wst")
        nc.vector.memset(warm[:], 0)
        nc.vector.memset(wst[:], 0)
        pw = wppool.tile([128, NFREE], f32, tag="pw")
        for w in range(N_WARM):
            pos = (0, 0) if w % 2 == 0 else (0, 64)
            dst = pw[0:56] if w % 2 == 0 else pw[64:120]
            nc.tensor.matmul(dst, wst[:], warm[:], start=True, stop=True,
                             tile_position=pos)

        # ---- main: 6 block pairs ----
        for q in range(NPAIR):
            xm_t, xi_t = xm_ts[q], xi_ts[q]
            pvh = [psum_pool.tile([128, NFREE], f32, tag="pe", name=f"pe{q}"),
                   psum_pool.tile([128, NFREE], f32, tag="po", name=f"po{q}")]
            pi = psum_pool.tile([128, NFREE], f32, tag="pi", name=f"pi{q}")
            for b in (0, 1):
                pt = pvh[b]
                bi = 2 * q + b
                for j in range(NJM):
                    nc.tensor.matmul(pt[0:56], opm_t[:, bi, j, 0:56],
                                     xm_t[:, 0, b, j, :],
                                     start=(j == 0), stop=False,
                                     tile_position=(0, 0))
                    nc.tensor.matmul(pt[64:120], opm_t[:, bi, j, 56:112],
                                     xm_t[:, 1, b, j, :],
                                     start=(j == 0), stop=False,
                                     tile_position=(0, 64))
                nc.tensor.matmul(pt[0:56], opt_t[:, bi, 0:56],
                                 xt_t[:, q, 0, b, :],
                                 start=False, stop=True,
                                 tile_position=(0, 0))
                nc.tensor.matmul(pt[64:120], opt_t[:, bi, 56:112],
                                 xt_t[:, q, 1, b, :],
                                 start=False, stop=True,
                                 tile_position=(0, 64))
            for c in range(2):
                kk = 128 if c == 0 else 96  # identity rows: 224 = 128 + 96
                nc.tensor.matmul(pi[0:56], opi_t[0:kk, 2 * q, c, :],
                                 xi_t[0:kk, 0, c, :],
                                 start=(c == 0), stop=(c == 1),
                                 tile_position=(0, 0))
                nc.tensor.matmul(pi[64:120], opi_t[0:kk, 2 * q + 1, c, :],
                                 xi_t[0:kk, 1, c, :],
                                 start=(c == 0), stop=(c == 1),
                                 tile_position=(0, 64))
            # drain psums -> bf16 staging -> one DMA per pair (ring A)
            stg = spool.tile([120, 3, NFREE], bf16, tag="stg", name=f"stg{q}")
            nc.scalar.copy(stg[:, 0, :], pvh[0][0:120])
            nc.vector.tensor_copy(stg[:, 1, :], pvh[1][0:120])
            nc.vector.tensor_copy(stg[0:56, 2, :], pi[0:56])
            nc.scalar.copy(stg[64:120, 2, :], pi[64:120])
            nc.scalar.dma_start(out_d[q], stg[:])
    nc.finalize()
    return nc


def prepare_inputs(x, w1, w2, w3, pad_hv, idx_identit):
    """Host-side shard prep. Returns in_maps (list of 8 dicts)."""
    x = np.asarray(x)
    xq = x.astype(F8)                                     # (B, C, 60, 60)
    opv, oph = _build_vh_operators(w1, w2, pad_hv)        # (96, 660, 56) f32
    ks, cf = _identity_slots(w3, idx_identit)             # (96,4) each

    in_maps = []
    for i in range(N_CORES):
        blocks = np.arange(i * BPC, (i + 1) * BPC)
        csl = slice(i * CPC, (i + 1) * CPC)
        # h-major rows (k,h'), cols (b,w); w-major rows (k,w'), cols (b,h)
        ch = xq[:, csl, :, EP:EP + WOUT]                   # (8, 132, 60, 56)
        ch = ch.transpose(1, 2, 0, 3).reshape(BPC, KROWS, NFREE)
        cw = xq[:, csl, EP:EP + HOUT, :]                   # (8, 132, 56, 60)
        cw = cw.transpose(1, 3, 0, 2).reshape(BPC, KROWS, NFREE)
        xall = np.stack([ch, cw], axis=1)                  # (12, 2, 660, 448)
        main = xall[:, :, :NJM * KM].reshape(BPC, 2, NJM, KM, NFREE)
        # xm: [pair, p, orient, blk, chunk, n]
        xm = np.ascontiguousarray(
            main.reshape(NPAIR, 2, 2, NJM, KM, NFREE)
                .transpose(0, 4, 2, 1, 3, 5))
        # xt: [p(20), pair, orient, blk, n]
        tail = xall[:, :, NJM * KM:]                       # (12, 2, 20, 448)
        xt = np.ascontiguousarray(
            tail.reshape(NPAIR, 2, 2, KT, NFREE).transpose(3, 0, 2, 1, 4))
        # identity gather: rows (slot, h'), 224 -> chunks 128 + 96(+32 pad)
        chan = (blocks[:, None] * NK + ks[blocks]).astype(np.int64)  # (12,4)
        xi_raw = xq[:, chan, EP:EP + HOUT, EP:EP + WOUT]   # (8, 12, 4, 56, 56)
        xi_raw = xi_raw.transpose(1, 2, 3, 0, 4).reshape(BPC, 224, NFREE)
        xi_pad = np.zeros((BPC, 2, KM, NFREE), F8)
        xi_pad[:, 0] = xi_raw[:, :KM]
        xi_pad[:, 1, :224 - KM] = xi_raw[:, KM:]
        xi = np.ascontiguousarray(
            xi_pad.reshape(NPAIR, 2, 2, KM, NFREE).transpose(0, 3, 1, 2, 4))
        # V/H operators: chunk rows match xm/xt row split
        opvh = np.concatenate([opv[blocks], oph[blocks]], axis=2)  # (12,660,112)
        opm = np.ascontiguousarray(
            opvh[:, :NJM * KM].reshape(BPC, NJM, KM, 112)
                 .transpose(2, 0, 1, 3)).astype(F8)        # (128, 12, 5, 112)
        opt = np.ascontiguousarray(
            opvh[:, NJM * KM:].transpose(1, 0, 2)).astype(F8)  # (20, 12, 112)
        # identity operators: (12, 224, 56) scaled-identity bands -> 128+96
        eye = np.eye(56, dtype=np.float32)
        bands = (cf[blocks][:, :, None, None] * eye).reshape(BPC, 224, 56)
        opi_pad = np.zeros((BPC, 2, KM, 56), np.float32)
        opi_pad[:, 0] = bands[:, :KM]
        opi_pad[:, 1, :224 - KM] = bands[:, KM:]
        opi = np.ascontiguousarray(
            opi_pad.transpose(2, 0, 1, 3)).astype(F8)      # (128, 12, 2, 56)
        in_maps.append({"xm": xm, "xt": xt, "xi": xi,
                        "opm": opm, "opt": opt, "opi": opi})
    return in_maps


def unshard(results):
    """-> (out_h, out_v, out_i) each (B, C_OUT, 56, 56) fp32."""
    o = np.stack([np.asarray(r["out"], np.float32) for r in results])
    # o: (8, NPAIR, 120, 3, 448); col 0 = blk_e, 1 = blk_o, 2 = identity
    vh = o[:, :, :, 0:2].transpose(0, 1, 3, 2, 4)  # (8, 6, 2, 120, 448)
    vh = vh.reshape(N_CORES, BPC, 120, NFREE)
    V = vh[:, :, 0:56].reshape(N_CORES, BPC, 56, B, WOUT)
    out_v = V.transpose(3, 0, 1, 2, 4).reshape(B, C_OUT, HOUT, WOUT)
    Hh = vh[:, :, 64:120].reshape(N_CORES, BPC, 56, B, HOUT)  # [.., w, b, h]
    out_h = Hh.transpose(3, 0, 1, 4, 2).reshape(B, C_OUT, HOUT, WOUT)
    ii = o[:, :, :, 2]                             # (8, 6, 120, 448)
    Ie = ii[:, :, 0:56].reshape(N_CORES, NPAIR, 56, B, WOUT)
    Io = ii[:, :, 64:120].reshape(N_CORES, NPAIR, 56, B, WOUT)
    I2 = np.stack([Ie, Io], axis=2)                # [core, pair, half, h, b, w]
    out_i = I2.transpose(4, 0, 1, 2, 3, 5).reshape(B, C_OUT, HOUT, WOUT)
    return out_h, out_v, out_i


def kernel(x, w1, w2, w3, pad_hv, idx_identit, b=B, hout=HOUT, wout=WOUT):
    from concourse.bass_utils import run_bass_kernel_spmd

    assert int(b) == B and int(hout) == HOUT and int(wout) == WOUT
    assert tuple(np.asarray(x).shape) == (B, C_IN, HIN, WIN)

    in_maps = prepare_inputs(x, w1, w2, w3, pad_hv, idx_identit)
    nc = _CACHE.get("nc")
    if nc is None:
        nc = _build_nc()
        _CACHE["nc"] = nc
    res = run_bass_kernel_spmd(nc, in_maps, core_ids=list(range(N_CORES)))
    return unshard(res.results)
